# revision 6
# baseline (speedup 1.0000x reference)
"""CrossModalityAttention Trainium2 kernel — fp8 DoubleRow projections.

Full inputs -> full output; internally shards batch B=8192 across 8 NeuronCores
(pure data parallel). Per core: 1024 samples x K=8 modalities = 8192 tokens of
D=1024.

Device strategy (per core), evolved from the bf16/f32r baseline:
  - All four DxD projections (Q/K/V/O) run as fp8-e4m3 DoubleRow matmuls:
    one instruction contracts TWO 128-chunks at 0.5 cycles/row -> 4x bf16 PE
    throughput. Weights are host-quantized as fp8(64*W^T) (64x scale keeps
    them out of e4m3's subnormal range); X is host-quantized to fp8 d-major.
    fp8 tensors travel as uint8 and are bitcast to float8e4 at the matmul.
  - Scale bookkeeping: Q psum = 64q -> ACT copy with scale 1/64 (+bq). K psum
    -> scale 1/(64*sqrt(128)) (+bk/sqrt(128)), folding the attention scale.
    V psum = 64v kept as bf16 (v64). O = exp-weighted v64 / Z = 64*O_norm,
    PE-transposed then ACT-copied to fp8 with scale 1/16 => ot = 4*O_norm.
    Output projection psum = (4*O)(64*wo) = 256*y; the residual XB is
    host-scaled 256*(x + bo + wo@bv), so res_psum = 256*(x+y). LayerNorm is
    scale-invariant; only eps must scale by 256^2.
  - Attention smalls all bf16 (probs, v, eye, ones) -> 1 cycle/row on PE.
  - Scores are computed transposed per 128-token group: ST[(s,k),(s',q)] via
    matmul(lhsT=Kh^T, rhs=Qh^T); off-diagonal sample pairs get -30 from the
    prior/mask table so exp() kills them; softmax normalization is deferred:
    O = exp(ST).T @ v64 token-major, Z via ones-matmul, O *= 1/Z per
    partition. rstd = exp(-0.5*ln(var+eps)) keeps ACT in one table set.
"""

import math

import numpy as np

import concourse.bacc as bacc
import concourse.bass as bass
import concourse.mybir as mybir
import concourse.tile as tile
from concourse.bass_utils import run_bass_kernel_spmd

N_CORES = 8
B, K, D = 8192, 8, 1024
H, HD = 8, 128
BC = B // N_CORES            # samples per core
T = BC * K                   # tokens per core (8192)
TS = 512                     # tokens per tile
NT = T // TS                 # tiles per core
GROUPS = TS // 128           # 128-token groups per tile
SPG = 128 // K               # samples per group (16)
LN_EPS = 1e-5
NEG = -30.0                  # large-negative mask for cross-sample scores
RS = 256.0                   # residual scale absorbed by LayerNorm

F32 = mybir.dt.float32
BF16 = mybir.dt.bfloat16
F8 = mybir.dt.float8e4
U8 = mybir.dt.uint8

_CACHED = None  # compiled Bacc module, built once per process


def _build():
    nc = bacc.Bacc("TRN2", target_bir_lowering=False, debug=False, num_devices=1)

    x8_d = nc.dram_tensor("X8", [D, T], U8, kind="ExternalInput").ap()
    xb_d = nc.dram_tensor("XB", [T, D], F32, kind="ExternalInput").ap()
    wq_d = nc.dram_tensor("WQ8", [D, D], U8, kind="ExternalInput").ap()
    wk_d = nc.dram_tensor("WK8", [D, D], U8, kind="ExternalInput").ap()
    wv_d = nc.dram_tensor("WV8", [D, D], U8, kind="ExternalInput").ap()
    wo_d = nc.dram_tensor("WO8", [D, D], U8, kind="ExternalInput").ap()
    bqk_d = nc.dram_tensor("BQK", [128, 2 * H], F32, kind="ExternalInput").ap()
    pm_d = nc.dram_tensor("PM", [128, 128], F32, kind="ExternalInput").ap()
    eye_d = nc.dram_tensor("EYE", [128, 128], F32, kind="ExternalInput").ap()
    ones_d = nc.dram_tensor("ONES1", [128, 1], BF16, kind="ExternalInput").ap()
    out_d = nc.dram_tensor("OUT", [T, D], F32, kind="ExternalOutput").ap()

    x8_r = x8_d.rearrange("(c p) t -> p c t", p=128)   # [128, 8, T]
    qs = 1.0 / 64.0
    ks = 1.0 / (64.0 * math.sqrt(HD))

    with tile.TileContext(nc) as tc:
        with (
            tc.tile_pool(name="wpool", bufs=1) as wpool,
            tc.tile_pool(name="consts", bufs=1) as consts,
            tc.tile_pool(name="x8p", bufs=2) as x8p,
            tc.tile_pool(name="qkp", bufs=2) as qkp,
            tc.tile_pool(name="vp", bufs=2) as vp,
            tc.tile_pool(name="ptp", bufs=2) as ptp,
            tc.tile_pool(name="osbp", bufs=2) as osbp,
            tc.tile_pool(name="otp", bufs=2) as otp,
            tc.tile_pool(name="xbp", bufs=2) as xbp,
            tc.tile_pool(name="smalls", bufs=4) as smalls,
            tc.tile_pool(name="projps", bufs=2, space="PSUM") as projps,
            tc.tile_pool(name="attps", bufs=2, space="PSUM") as attps,
            tc.tile_pool(name="zps", bufs=2, space="PSUM") as zps,
        ):
            # ---- constants / weights (resident) ----
            wq = wpool.tile([128, 8, D], U8, tag="w_q")
            nc.sync.dma_start(wq[:], wq_d.rearrange("(c p) m -> p c m", p=128))
            wk = wpool.tile([128, 8, D], U8, tag="w_k")
            nc.sync.dma_start(wk[:], wk_d.rearrange("(c p) m -> p c m", p=128))
            wv = wpool.tile([128, 8, D], U8, tag="w_v")
            nc.sync.dma_start(wv[:], wv_d.rearrange("(c p) m -> p c m", p=128))
            wo = wpool.tile([128, 8, D], U8, tag="w_o")
            nc.sync.dma_start(wo[:], wo_d.rearrange("(c p) m -> p c m", p=128))
            bqk = consts.tile([128, 2 * H], F32)
            nc.sync.dma_start(bqk[:], bqk_d)
            pm = consts.tile([128, 128], F32)
            nc.sync.dma_start(pm[:], pm_d)
            eye = consts.tile([128, 128], F32)
            nc.sync.dma_start(eye[:], eye_d)
            ones1 = consts.tile([128, 1], BF16)
            nc.sync.dma_start(ones1[:], ones_d)
            eps = consts.tile([128, 1], F32)
            nc.vector.memset(eps[:], LN_EPS * RS * RS)

            for t in range(NT):
                tok0 = t * TS
                x8 = x8p.tile([128, 8, TS], U8)
                nc.sync.dma_start(x8[:], x8_r[:, :, tok0 : tok0 + TS])

                # ---- Q^T, K^T projections (fp8 DoubleRow): [128 x TS]
                qt = qkp.tile([128, H, TS], BF16, tag="qt")
                kt = qkp.tile([128, H, TS], BF16, tag="kt")
                for wt, dst, bias_col0, sc in (
                    (wq, qt, 0, qs), (wk, kt, H, ks)
                ):
                    for h in range(H):
                        ps = projps.tile([128, TS], F32, tag="projps")
                        for c in range(4):
                            nc.tensor.matmul(
                                ps[:],
                                wt[:, 2 * c : 2 * c + 2, h * HD : (h + 1) * HD].bitcast(F8),
                                x8[:, 2 * c : 2 * c + 2, :].bitcast(F8),
                                start=(c == 0),
                                stop=(c == 3),
                                perf_mode=mybir.MatmulPerfMode.DoubleRow,
                            )
                        nc.scalar.activation(
                            dst[:, h, :],
                            ps[:],
                            mybir.ActivationFunctionType.Identity,
                            bias=bqk[:, bias_col0 + h : bias_col0 + h + 1],
                            scale=sc,
                        )

                # ---- V projection (fp8 DoubleRow), token-major; psum = 64*v
                v = vp.tile([128, GROUPS, H, HD], BF16, tag="v")
                for sub in range(GROUPS):
                    for half in range(2):
                        psv = projps.tile([128, 512], F32, tag="projps")
                        for c in range(4):
                            nc.tensor.matmul(
                                psv[:],
                                x8[:, 2 * c : 2 * c + 2, sub * 128 : (sub + 1) * 128].bitcast(F8),
                                wv[:, 2 * c : 2 * c + 2, half * 512 : (half + 1) * 512].bitcast(F8),
                                start=(c == 0),
                                stop=(c == 3),
                                perf_mode=mybir.MatmulPerfMode.DoubleRow,
                            )
                        nc.vector.tensor_copy(
                            v[:, sub, 4 * half : 4 * half + 4, :],
                            psv.rearrange("p (a b) -> p a b", a=4),
                        )

                # ---- attention + output proj + residual + LN per 128-tok group
                mvt = smalls.tile([128, GROUPS, 2], F32, tag="mvt")
                xbt = xbp.tile([128, GROUPS, D], F32)
                nc.sync.dma_start(
                    xbt[:],
                    xb_d[tok0 : tok0 + TS, :].rearrange("(g p) d -> p g d", p=128),
                )
                for g in range(GROUPS):
                    gsl = slice(g * 128, (g + 1) * 128)
                    st = attps.tile([128, H, 128], F32, tag="attps")
                    for h in range(H):
                        nc.tensor.matmul(st[:, h, :], kt[:, h, gsl], qt[:, h, gsl])
                    # add prior/mask (same [128,128] table per head), in place
                    nc.vector.tensor_tensor(
                        st[:],
                        st[:],
                        pm[:, None, :].to_broadcast((128, H, 128)),
                        mybir.AluOpType.add,
                    )
                    pt = ptp.tile([128, H, 128], BF16)
                    nc.scalar.activation(
                        pt[:], st[:], mybir.ActivationFunctionType.Exp
                    )
                    oz = attps.tile([128, H, 128], F32, tag="attps")
                    zp = zps.tile([128, H], F32)
                    for h in range(H):
                        nc.tensor.matmul(oz[:, h, :], pt[:, h, :], v[:, g, h, :])
                        nc.tensor.matmul(zp[:, h : h + 1], pt[:, h, :], ones1[:])
                    rz = smalls.tile([128, H], F32, tag="rz")
                    nc.vector.reciprocal(rz[:], zp[:])
                    osb = osbp.tile([128, H, HD], F32)
                    nc.vector.tensor_tensor(
                        osb[:],
                        oz[:],
                        rz[:, :, None].to_broadcast((128, H, HD)),
                        mybir.AluOpType.mult,
                    )
                    tp = attps.tile([128, H, 128], F32, tag="attps")
                    for h in range(H):
                        nc.tensor.transpose(tp[:, h, :], osb[:, h, :], eye[:])
                    # ot = tp/16 = 4*O_norm in fp8 (tp = 64*O_norm)
                    ot = otp.tile([128, H, 128], F8)
                    nc.scalar.activation(
                        ot[:], tp[:], mybir.ActivationFunctionType.Copy,
                        scale=1.0 / 16.0,
                    )

                    xb = xbt[:, g, :]
                    for half in range(2):
                        yp = projps.tile([128, 512], F32, tag="projps")
                        for c in range(4):
                            nc.tensor.matmul(
                                yp[:],
                                ot[:, 2 * c : 2 * c + 2, :],
                                wo[:, 2 * c : 2 * c + 2, half * 512 : (half + 1) * 512].bitcast(F8),
                                start=(c == 0),
                                stop=(c == 3),
                                perf_mode=mybir.MatmulPerfMode.DoubleRow,
                            )
                        nc.vector.tensor_tensor(
                            xb[:, half * 512 : (half + 1) * 512],
                            xb[:, half * 512 : (half + 1) * 512],
                            yp[:],
                            mybir.AluOpType.add,
                        )
                    stats = smalls.tile([128, 2, 6], F32, tag="stats")
                    for sg in range(2):
                        nc.vector.bn_stats(
                            stats[:, sg, :], xb[:, sg * 512 : (sg + 1) * 512]
                        )
                    nc.vector.bn_aggr(mvt[:, g, :], stats[:])

                # rstd = exp(-0.5*ln(var+eps)) for all 4 groups in one ln/exp
                # pair per tile: 4x fewer ACT table switches vs per-group
                sd4 = smalls.tile([128, GROUPS, 1], F32, tag="sd4")
                nc.scalar.activation(
                    sd4[:],
                    mvt[:, :, 1:2],
                    mybir.ActivationFunctionType.Ln,
                    bias=eps[:],
                )
                nc.scalar.activation(
                    sd4[:], sd4[:], mybir.ActivationFunctionType.Exp, scale=-0.5
                )
                for g in range(GROUPS):
                    nc.vector.tensor_scalar(
                        out=xbt[:, g, :],
                        in0=xbt[:, g, :],
                        scalar1=mvt[:, g, 0:1],
                        scalar2=sd4[:, g, :],
                        op0=mybir.AluOpType.subtract,
                        op1=mybir.AluOpType.mult,
                    )
                nc.sync.dma_start(
                    out_d[tok0 : tok0 + TS, :].rearrange("(g p) d -> p g d", p=128),
                    xbt[:],
                )

    nc.compile()
    return nc


def _get_nc():
    global _CACHED
    if _CACHED is None:
        _CACHED = _build()
    return _CACHED


def _reference_numpy(modality_encodings, selection_mask, wq, bq, wk, bk, wv, bv,
                     wo, bo, rel_prior, ln_gamma, ln_beta):
    """Slow fallback, exact port of the reference (used only if inputs fall
    outside the fast path's assumptions: non-trivial mask)."""
    x = modality_encodings.astype(np.float32)
    Bn, Kn, Dn = x.shape
    Hd = Dn // H
    q = (x @ wq.T + bq).reshape(Bn, Kn, H, Hd).transpose(0, 2, 1, 3)
    k = (x @ wk.T + bk).reshape(Bn, Kn, H, Hd).transpose(0, 2, 1, 3)
    v = (x @ wv.T + bv).reshape(Bn, Kn, H, Hd).transpose(0, 2, 1, 3)
    scores = np.einsum("bhqd,bhkd->bhqk", q, k) / math.sqrt(Hd)
    scores = scores + rel_prior[None, None]
    mask2d = (selection_mask[:, :, None] * selection_mask[:, None, :]) > 0
    scores = np.where(mask2d[:, None], scores, -np.inf)
    scores = scores - scores.max(axis=-1, keepdims=True)
    e = np.exp(scores)
    attn = e / e.sum(axis=-1, keepdims=True)
    out = np.einsum("bhqk,bhkd->bhqd", attn, v)
    out = out.transpose(0, 2, 1, 3).reshape(Bn, Kn, Dn)
    out = out @ wo.T + bo
    res = x + out
    mu = res.mean(-1, keepdims=True)
    var = ((res - mu) ** 2).mean(-1, keepdims=True)
    return (res - mu) / np.sqrt(var + LN_EPS) * ln_gamma + ln_beta


def _prep_in_maps(modality_encodings, wq, bq, wk, bk, wv, bv, wo, bo, rel_prior):
    import ml_dtypes

    s = 1.0 / math.sqrt(HD)

    def q8(w):  # [out,in] nn.Linear weight -> fp8(64*W^T) as uint8
        return np.ascontiguousarray((w * 64.0).T).astype(
            ml_dtypes.float8_e4m3).view(np.uint8)

    wq8, wk8, wv8, wo8 = q8(wq), q8(wk), q8(wv), q8(wo)
    b_eff = (bo + wo @ bv).astype(np.float32)

    bqk = np.concatenate(
        [bq.reshape(H, HD).T, (bk * s).reshape(H, HD).T], axis=1
    ).astype(np.float32)  # [128, 16]

    pmat = np.full((128, 128), NEG, dtype=np.float32)
    for sm in range(SPG):
        pmat[sm * K : (sm + 1) * K, sm * K : (sm + 1) * K] = rel_prior.T
    eye = np.eye(128, dtype=np.float32)
    ones1 = np.ones((128, 1), dtype=np.float32).astype(ml_dtypes.bfloat16)

    x_flat = modality_encodings.reshape(B * K, D)
    in_maps = []
    for c in range(N_CORES):
        x_core = x_flat[c * T : (c + 1) * T]
        xt8 = np.ascontiguousarray(x_core.T).astype(
            ml_dtypes.float8_e4m3).view(np.uint8)
        in_maps.append({
            "X8": xt8,
            "XB": RS * (x_core + b_eff),
            "WQ8": wq8, "WK8": wk8, "WV8": wv8, "WO8": wo8,
            "BQK": bqk, "PM": pmat, "EYE": eye, "ONES1": ones1,
        })
    return in_maps


def run_device(inputs, trace=False):
    """Build in_maps from full inputs, run on 8 cores, return (full_out, results)."""
    in_maps = _prep_in_maps(
        inputs["modality_encodings"], inputs["wq"], inputs["bq"], inputs["wk"],
        inputs["bk"], inputs["wv"], inputs["bv"], inputs["wo"], inputs["bo"],
        inputs["rel_prior"],
    )
    nc = _get_nc()
    res = run_bass_kernel_spmd(nc, in_maps, core_ids=list(range(N_CORES)), trace=trace)
    out = np.concatenate(
        [res.results[c]["OUT"].reshape(BC, K, D) for c in range(N_CORES)], axis=0
    )
    return out, res


def kernel(**inputs) -> np.ndarray:
    inputs = {k: np.asarray(v) for k, v in inputs.items()}
    mask = inputs["selection_mask"]
    gamma = inputs["ln_gamma"]
    beta = inputs["ln_beta"]
    if not np.all(mask > 0):
        # general-mask fallback (never hit for the spec'd inputs: fill=ones)
        return _reference_numpy(**{k: inputs[k].astype(np.float32) for k in (
            "modality_encodings", "selection_mask", "wq", "bq", "wk", "bk",
            "wv", "bv", "wo", "bo", "rel_prior", "ln_gamma", "ln_beta")}
        ).astype(np.float32)

    out, _ = run_device(inputs, trace=False)
    # device kernel skips the (identity for spec'd inputs) LN affine params
    if not (np.all(gamma == 1.0) and np.all(beta == 0.0)):
        out = out * gamma + beta
    return out.astype(np.float32)
